# revision 1
# baseline (speedup 1.0000x reference)
"""Trainium2 Bass kernel for nn_LossNet_42494406426743 (contrastive loss_fn).

Math (reference, temp=0.1, B=4096):
    xn = l2_normalize(x); xe, ye, ze = split(xn, 3)
    For pairs (a,b) in {xx, yy, xy, xz, yz(+transposes zx, zy)}:
        d_ab[i] = exp(a_i.b_i/t)  (diagonal)
        s_ab[i] = sum_j exp(a_i.b_j/t)  (row sums of the exp-similarity matrix)
    loss = mean_{ij}[-2 log(d_xy[j]/((S[i]-D[j])))] + 4 aux terms of
           mean_{ij}[-log(d[j]/(s[i]-d[j]))]

Device work (sharded 8 ways over rows; each core owns 256 "low" + 256 "high"
rows of each of xe and ye; z never appears as a row operand):
    - bf16 matmuls vs the SBUF-resident full embedding matrix (stationary
      operand = own row chunks, streaming operand = shared rhsT)
    - ScalarE exp (scale=1/t folded in) with fused accum_out row-sums -- the
      only engine that can do exp, so everything is shaped to keep it busy
    - exp(XZ^T), exp(YZ^T) tiles are also column-reduced (colacc + tiny
      stationary-matmuls vs a ones vector) to recover the zx / zy row sums
      without re-exponentiating the transposed matrices
    - XX and YY exploit symmetry: low rows compute full rows; high rows
      compute only the right half directly and take the left half from the
      transposed column sums of the low rows' right half (saves 10% of exp)
Host work (O(B), fp64): diagonals, assembling s vectors, and the
mean_{ij} log(s[i]-d[j]) terms evaluated exactly via a binomial power-series
factorization (O(B*K) instead of O(B^2); exact fallback if out of range).
"""

import numpy as np
import ml_dtypes

_BF16 = ml_dtypes.bfloat16

# Problem constants (hardcoded per harness contract).
_N = 12288          # total rows
_D = 128            # feature dim
_B = 4096           # rows per split
_NCORES = 8
_R = _B // _NCORES  # 512 own rows per split per core
_TEMP = 0.1
_EPS = 1e-12

_STATE = {}


def _build_nc(T=1):
    import concourse.bacc as bacc
    import concourse.mybir as mybir
    import concourse.tile as tile

    f32 = mybir.dt.float32
    bf16 = mybir.dt.bfloat16
    Exp = mybir.ActivationFunctionType.Exp

    nc = bacc.Bacc("TRN2")
    # Inputs: own rows (512 x-rows then 512 y-rows), pre-transposed; full
    # embedding matrix pre-transposed (feature dim on partitions).
    lhsT = nc.dram_tensor("lhsT", [128, 2 * _R], bf16, kind="ExternalInput")
    rhsT = nc.dram_tensor("rhsT", [128, _N], bf16, kind="ExternalInput")
    # Outputs: 37 accum slots (row-sum partials) + column-sum partials for
    # zx (32 chunks), zy (32), xxB (16), yyB (16):
    # out_cs[p, base+ch] = colsum of that accumulator's column ch*128+p.
    out_s = nc.dram_tensor("out_s", [128, 37], f32, kind="ExternalOutput")
    out_cs = nc.dram_tensor("out_cs", [128, 96], f32, kind="ExternalOutput")

    G = 2048  # columns per ACT group (4 psum banks)

    with tile.TileContext(nc) as tc:
        with (
            tc.tile_pool(name="singles", bufs=1) as singles,
            tc.tile_pool(name="etp", bufs=3) as etp,
            tc.tile_pool(name="ps", bufs=2, space="PSUM") as ps,
        ):
            lhsT_t = singles.tile([128, 2 * _R], bf16)
            rhsT_t = singles.tile([128, _N], bf16)
            ones_t = singles.tile([128, 1], bf16)
            act_warm = singles.tile([128, 1], f32)
            s_acc = singles.tile([128, 37], f32)
            colacc_zx = singles.tile([128, _B], bf16)
            colacc_zy = singles.tile([128, _B], bf16)
            colacc_xxB = singles.tile([128, G], bf16)
            colacc_yyB = singles.tile([128, G], bf16)
            cs_sbuf = singles.tile([128, 96], f32)

            nc.vector.memset(ones_t[:], 1.0)
            # Pull the exp ACT-table load into the input-DMA shadow.
            nc.scalar.activation(act_warm[:], ones_t[:], Exp, scale=1.0)
            # lhsT rides the GPSIMD SWDGE queue so it lands in parallel with
            # the rhs stream on the SP HWDGE queue.
            nc.gpsimd.dma_start(lhsT_t[:, 0:128], lhsT[:, 0:128])
            nc.sync.dma_start(rhsT_t[:, 0:1024], rhsT[:, 0:1024])
            nc.gpsimd.dma_start(lhsT_t[:, 128:1024], lhsT[:, 128:1024])
            nc.sync.dma_start(rhsT_t[:, 1024:2048], rhsT[:, 1024:2048])
            for p in range(1, _N // G):
                nc.sync.dma_start(rhsT_t[:, p * G:(p + 1) * G], rhsT[:, p * G:(p + 1) * G])

            colaccs = {"zx": colacc_zx, "zy": colacc_zy,
                       "xxB": colacc_xxB, "yyB": colacc_yyB}
            for _t in range(T):
                _emit_body(nc, tc, etp, ps, lhsT_t, rhsT_t, ones_t, s_acc,
                           colaccs, cs_sbuf, _t)

            nc.sync.dma_start(out_s[:], s_acc[:])
            nc.sync.dma_start(out_cs[:], cs_sbuf[:])

    nc.finalize()
    return nc


# Per m-chunk: (stream start column, number of 2048-col groups).
# m0,m1 = "low" x rows, m2,m3 = "high" x rows, m4,m5 = low y, m6,m7 = high y.
# Low rows compute their symmetric block fully; high rows compute only the
# right half (cols [2048,4096) x-local) and recover the left half from the
# transposed colsums of the low rows' right half (xxB / yyB accumulators).
_CHUNK_SPECS = [
    (0, 6), (0, 6),        # low x:  XX-L, XX-R, XY, XY, XZ, XZ
    (2048, 5), (2048, 5),  # high x: XX-R, XY, XY, XZ, XZ
    (4096, 4), (4096, 4),  # low y:  YY-L, YY-R, YZ, YZ
    (6144, 3), (6144, 3),  # high y: YY-R, YZ, YZ
]
def _colacc_plan(m, col0):
    """Return (key, dst_off, is_first) if group at global col0 feeds a
    column accumulator, else None."""
    if col0 >= 8192:  # z columns
        key = "zx" if m < 4 else "zy"
        return key, col0 - 8192, m in (0, 4)
    if m in (0, 1) and col0 == 2048:
        return "xxB", 0, m == 0
    if m in (4, 5) and col0 == 6144:
        return "yyB", 0, m == 4
    return None


def _emit_body(nc, tc, etp, ps, lhsT_t, rhsT_t, ones_t, s_acc,
               colaccs, cs_sbuf, t):
    import concourse.mybir as mybir

    f32 = mybir.dt.float32
    bf16 = mybir.dt.bfloat16
    Exp = mybir.ActivationFunctionType.Exp
    G = 2048

    def reduce_cs(keys, outmap, tag):
        # Partition-reduce column accumulators: colacc chunks as the
        # stationary operand vs a ones vector -> [128,1] colsums per chunk,
        # packed into one PSUM bank, evacuated with DVE copies into the
        # cs_sbuf layout given by outmap {key: dest col offset}.
        total = sum(colaccs[k].shape[1] // 128 for k in keys)
        cs_ps = ps.tile([128, total], f32, tag="mm", name=f"cs_{tag}_{t}")
        idx = 0
        spans = []
        for key in keys:
            nch = colaccs[key].shape[1] // 128
            for ch in range(nch):
                nc.tensor.matmul(
                    cs_ps[:, idx + ch:idx + ch + 1],
                    colaccs[key][:, ch * 128:(ch + 1) * 128],
                    ones_t[:],
                    start=True, stop=True,
                )
            spans.append((idx, nch, outmap[key]))
            idx += nch
        if all(i0 == o0 for i0, _, o0 in spans):
            nc.vector.tensor_copy(cs_sbuf[:, 0:idx], cs_ps[:, 0:idx])
        else:
            for i0, nch, o0 in spans:
                nc.vector.tensor_copy(cs_sbuf[:, o0:o0 + nch], cs_ps[:, i0:i0 + nch])

    slot = 0
    for m, (start, ngroups) in enumerate(_CHUNK_SPECS):
        lhs_chunk = lhsT_t[:, m * 128:(m + 1) * 128]
        # For the last chunk, stream the z-column groups first so colacc_zy
        # finishes early and the tail colsum-reduce overlaps the final exps.
        order = [1, 2, 0] if m == 7 else range(ngroups)
        # Split the very first group in half so the first exp op only waits
        # for 1024 columns of input (cuts the startup bubble).
        spans = []
        for g in order:
            if m == 0 and g == 0:
                spans += [(0, 1024, "0a"), (1024, 1024, "0b")]
            else:
                spans.append((start + g * G, G, str(g)))
        for col0, width, gname in spans:
            pt = ps.tile([128, width], f32, tag="mm", name=f"pt_{t}_{m}_{gname}")
            for k in range(width // 512):
                c0 = col0 + k * 512
                nc.tensor.matmul(
                    pt[:, k * 512:(k + 1) * 512],
                    lhs_chunk,
                    rhsT_t[:, c0:c0 + 512],
                    start=True, stop=True,
                )
            et = etp.tile([128, width], bf16, tag="et", name=f"et_{t}_{m}_{gname}")
            nc.scalar.activation(
                et[:], pt[:], Exp, scale=1.0 / _TEMP,
                accum_out=s_acc[:, slot:slot + 1],
            )
            slot += 1
            plan = _colacc_plan(m, col0)
            if plan is not None:
                key, off, first = plan
                dst = colaccs[key][:, off:off + G]
                if first:
                    nc.vector.tensor_copy(dst, et[:])
                else:
                    nc.vector.tensor_add(dst, dst, et[:])
    assert slot == 37
    # zy last: only its 32 reduce-matmuls gate on the final chunk's adds;
    # zx/xxB/yyB reduce while the y-phase exps still run.
    reduce_cs(("zx", "xxB", "yyB", "zy"),
              {"zx": 0, "xxB": 32, "yyB": 48, "zy": 64}, "all")


class _Exec:
    """Cached sharded-jit executor for the finalized Bass module (modeled on
    concourse.bass2jax.run_bass_via_pjrt, but reusable across calls)."""

    def __init__(self, nc, n_cores):
        import jax
        import concourse.mybir as mybir
        from concourse import bass2jax
        from jax.sharding import Mesh, PartitionSpec
        from jax.experimental.shard_map import shard_map

        bass2jax.install_neuronx_cc_hook()
        self._jax = jax
        self.nc = nc
        self.n_cores = n_cores
        partition_name = (
            nc.partition_id_tensor.name if nc.partition_id_tensor else None
        )
        in_names, out_names, out_avals, zero_outs = [], [], [], []
        for alloc in nc.m.functions[0].allocations:
            if not isinstance(alloc, mybir.MemoryLocationSet):
                continue
            name = alloc.memorylocations[0].name
            if alloc.kind == "ExternalInput":
                if name != partition_name:
                    in_names.append(name)
            elif alloc.kind == "ExternalOutput":
                shape = tuple(alloc.tensor_shape)
                dtype = mybir.dt.np(alloc.dtype)
                out_names.append(name)
                out_avals.append(jax.core.ShapedArray(shape, dtype))
                zero_outs.append(np.zeros(shape, dtype))
        self.in_names = list(in_names)
        self.out_names = out_names
        self.out_avals = out_avals
        self.zero_outs = zero_outs
        n_params = len(in_names)
        n_outs = len(out_names)
        bind_in_names = in_names + out_names + (
            [partition_name] if partition_name else []
        )

        def _body(*args):
            operands = list(args)
            if partition_name is not None:
                operands.append(bass2jax.partition_id_tensor())
            outs = bass2jax._bass_exec_p.bind(
                *operands,
                out_avals=tuple(out_avals),
                in_names=tuple(bind_in_names),
                out_names=tuple(out_names),
                lowering_input_output_aliases=(),
                sim_require_finite=True,
                sim_require_nnan=True,
                nc=nc,
            )
            return tuple(outs)

        devices = jax.devices()[:n_cores]
        assert len(devices) == n_cores
        self.mesh = Mesh(np.asarray(devices), ("core",))
        donate = tuple(range(n_params, n_params + n_outs))
        self.fn = jax.jit(
            shard_map(
                _body,
                mesh=self.mesh,
                in_specs=(PartitionSpec("core"),) * (n_params + n_outs),
                out_specs=(PartitionSpec("core"),) * n_outs,
                check_rep=False,
            ),
            donate_argnums=donate,
            keep_unused=True,
        )

    def make_zeros(self):
        return [
            np.zeros((self.n_cores * z.shape[0], *z.shape[1:]), z.dtype)
            for z in self.zero_outs
        ]

    def concat_inputs(self, in_maps):
        return [
            np.concatenate([np.asarray(in_maps[c][n]) for c in range(self.n_cores)], axis=0)
            for n in self.in_names
        ]

    def run_raw(self, concat_in, zeros):
        return self.fn(*concat_in, *zeros)

    def __call__(self, in_maps):
        out_arrs = self.fn(*self.concat_inputs(in_maps), *self.make_zeros())
        res = []
        for c in range(self.n_cores):
            res.append({
                name: np.asarray(out_arrs[i]).reshape(
                    self.n_cores, *self.out_avals[i].shape)[c]
                for i, name in enumerate(self.out_names)
            })
        return res


def _get_exec(T=1):
    key = ("exec", T)
    if key not in _STATE:
        nc = _build_nc(T)
        _STATE[key] = _Exec(nc, _NCORES)
    return _STATE[key]


def _mlod_exact(s, d):
    """mean_{ij} log(s[i] - d[j]) computed directly (chunked)."""
    tot = 0.0
    for i0 in range(0, s.shape[0], 256):
        tot += float(np.log(np.subtract.outer(s[i0:i0 + 256], d)).sum())
    return tot / (s.shape[0] * d.shape[0])


def _mlod(s, d):
    """mean_{ij} log(s[i] - d[j]) via binomial power-series factorization.

    log(s_i - d_j) = log M + log1p(u_i - v_j) with M = mean(s) - mean(d),
    u = (s-mean(s))/M, v = (d-mean(d))/M.  mean_{ij} (u_i-v_j)^k factorizes
    into products of power means, so the double mean is O(B*K).
    """
    from math import comb

    s = np.asarray(s, np.float64)
    d = np.asarray(d, np.float64)
    ms, md = s.mean(), d.mean()
    M = ms - md
    if not np.isfinite(M) or M <= 0:
        return _mlod_exact(s, d)
    u = (s - ms) / M
    v = (d - md) / M
    wmax = np.abs(u).max() + np.abs(v).max()
    if wmax > 0.5:
        return _mlod_exact(s, d)
    K = 120
    P = np.empty(K + 1)
    Q = np.empty(K + 1)
    up = np.ones_like(u)
    vp = np.ones_like(v)
    for k in range(K + 1):
        P[k] = up.mean()
        Q[k] = vp.mean()
        up *= u
        vp *= -v
    total = 0.0
    for k in range(1, K + 1):
        mk = 0.0
        for m in range(k + 1):
            mk += comb(k, m) * P[m] * Q[k - m]
        term = (1.0 if k % 2 == 1 else -1.0) / k * mk
        total += term
        if k > 6 and abs(term) < 1e-18 * max(1.0, abs(total)):
            break
    return float(np.log(M)) + total


def _host_prepare(x):
    """fp32 normalize (mirrors reference), bf16 cast, per-core device inputs."""
    x = np.asarray(x, np.float32)
    n = np.sqrt((x * x).sum(axis=1, keepdims=True))
    xn = x / np.maximum(n, _EPS)
    xnb = xn.astype(_BF16)
    rhsT = np.ascontiguousarray(xnb.T)  # [128, 12288]
    H = _B // 2
    in_maps = []
    for c in range(_NCORES):
        lo = c * 256
        rows = np.concatenate([
            xnb[lo:lo + 256],                    # low x  (m0, m1)
            xnb[H + lo:H + lo + 256],            # high x (m2, m3)
            xnb[_B + lo:_B + lo + 256],          # low y  (m4, m5)
            xnb[_B + H + lo:_B + H + lo + 256],  # high y (m6, m7)
        ], axis=0)
        in_maps.append({"lhsT": np.ascontiguousarray(rows.T), "rhsT": rhsT})
    return xn, in_maps


_SLOT0 = [0, 7, 13, 18, 23, 27, 31, 34]  # first accum slot of each m-chunk


def _assemble_s(results):
    """Decode device outputs into the seven s vectors (fp64)."""
    H = _B // 2
    s_xx = np.empty(_B)
    s_xy = np.empty(_B)
    s_ax = np.empty(_B)
    s_yy = np.empty(_B)
    s_ay = np.empty(_B)
    s_zx = np.zeros(_B)
    s_zy = np.zeros(_B)
    for c in range(_NCORES):
        sa = np.asarray(results[c]["out_s"], np.float64)  # [128, 36]
        for m in range(8):
            half = (m // 2) % 2            # 0 = low rows, 1 = high rows
            i0 = half * H + c * 256 + (m % 2) * 128
            s0 = _SLOT0[m]
            if m < 4:
                if half == 0 and m == 0:
                    # m0's first group is split in two: XX-La, XX-Lb, XX-R,
                    # XY, XY, XZ, XZ
                    s_xx[i0:i0 + 128] = sa[:, s0] + sa[:, s0 + 1] + sa[:, s0 + 2]
                    s_xy[i0:i0 + 128] = sa[:, s0 + 3] + sa[:, s0 + 4]
                    s_ax[i0:i0 + 128] = sa[:, s0 + 5] + sa[:, s0 + 6]
                elif half == 0:  # low x: XX-L, XX-R, XY, XY, XZ, XZ
                    s_xx[i0:i0 + 128] = sa[:, s0] + sa[:, s0 + 1]
                    s_xy[i0:i0 + 128] = sa[:, s0 + 2] + sa[:, s0 + 3]
                    s_ax[i0:i0 + 128] = sa[:, s0 + 4] + sa[:, s0 + 5]
                else:           # high x: XX-R, XY, XY, XZ, XZ
                    s_xx[i0:i0 + 128] = sa[:, s0]
                    s_xy[i0:i0 + 128] = sa[:, s0 + 1] + sa[:, s0 + 2]
                    s_ax[i0:i0 + 128] = sa[:, s0 + 3] + sa[:, s0 + 4]
            else:
                if half == 0:   # low y: YY-L, YY-R, YZ, YZ
                    s_yy[i0:i0 + 128] = sa[:, s0] + sa[:, s0 + 1]
                    s_ay[i0:i0 + 128] = sa[:, s0 + 2] + sa[:, s0 + 3]
                elif m == 6:    # high y: YY-R, YZ, YZ
                    s_yy[i0:i0 + 128] = sa[:, s0]
                    s_ay[i0:i0 + 128] = sa[:, s0 + 1] + sa[:, s0 + 2]
                else:       # m7 streams z-first: YZ, YZ, YY-R
                    s_yy[i0:i0 + 128] = sa[:, s0 + 2]
                    s_ay[i0:i0 + 128] = sa[:, s0] + sa[:, s0 + 1]
    # Column-sum contributions (after all direct assignments, since those
    # use `=` while these accumulate across every core).
    cs_sum = np.zeros((128, 96), np.float64)
    for c in range(_NCORES):
        cs_sum += np.asarray(results[c]["out_cs"], np.float64)
    # col idx base+ch holds colsums for accumulator column ch*128 + p
    # (layout: zx | xxB | yyB | zy)
    s_zx += cs_sum[:, 0:32].T.reshape(-1)
    s_xx[H:] += cs_sum[:, 32:48].T.reshape(-1)
    s_yy[H:] += cs_sum[:, 48:64].T.reshape(-1)
    s_zy += cs_sum[:, 64:96].T.reshape(-1)
    return s_xx, s_xy, s_ax, s_yy, s_ay, s_zx, s_zy


def _host_combine(xn, results):
    xe = xn[:_B].astype(np.float64)
    ye = xn[_B:2 * _B].astype(np.float64)
    ze = xn[2 * _B:].astype(np.float64)
    inv_t = 1.0 / _TEMP
    d_xx = np.exp((xe * xe).sum(1) * inv_t)
    d_yy = np.exp((ye * ye).sum(1) * inv_t)
    d_xy = np.exp((xe * ye).sum(1) * inv_t)
    d_ax = np.exp((xe * ze).sum(1) * inv_t)
    d_ay = np.exp((ye * ze).sum(1) * inv_t)

    s_xx, s_xy, s_ax, s_yy, s_ay, s_zx, s_zy = _assemble_s(results)

    S_mut = s_xy + s_xx + s_yy
    D_mut = d_xy + d_xx + d_yy
    loss_mutual = -2.0 * float(np.log(d_xy).mean()) + 2.0 * _mlod(S_mut, D_mut)

    def aux(d, s):
        return -float(np.log(d).mean()) + _mlod(s, d)

    loss = (loss_mutual + aux(d_ax, s_ax) + aux(d_ay, s_ay)
            + aux(d_ax, s_zx) + aux(d_ay, s_zy))
    return np.array(loss, dtype=np.float32)


def kernel(x):
    ex = _get_exec()
    xn, in_maps = _host_prepare(x)
    results = ex(in_maps)
    return _host_combine(xn, results)


if __name__ == "__main__":
    rng = np.random.default_rng(0)
    x = rng.standard_normal((_N, _D)).astype(np.float32)
    print(kernel(x))

